# revision 8
# baseline (speedup 1.0000x reference)
"""2-layer GraphSAGE on 8 trn2 NeuronCores - v3.

v1 dataflow (HBM-table dma_gather + one-hot scatter matmuls) plus:
  - gathers spread over 4 SWDGE queues (parallel descriptor streams; ~2-4x
    faster gather on HW),
  - xT supplied pre-transposed fp16 from host (no PE transposes in Phase A),
  - biases folded into ones x bias matmuls (no DVE broadcast adds),
  - layer-2 projection fused into the layer-1 aggregation block loop.

Strategy (graph/data parallel, hardcoded for N=50000, E=800000, D=128, 8 cores):
  - Nodes sharded by contiguous ranges of 6250 (padded to 6272 = 49*128) per core.
  - Host preprocesses edges: sorted by (dst core, dst block, src half, src),
    padded so every (block, half) has a uniform chunk count across cores (SPMD).
  - Device per layer:
      * project own rows: p = relu(x @ WpT + bp)  -> fp16, AllGather into a
        replicated [50176,128] fp16 table in DRAM.
      * dma_gather (SWDGE) message rows from the table (two int16-indexed
        halves), 128 edges per chunk.
      * scatter via one-hot matmuls: aggT[k,d] += msg[e,k]^T @ onehot[e,d],
        onehot built on DVE with is_equal against an iota tile.
      * mean via per-dst invdeg multiply, then output matmuls + bias (+relu).
  - Layer-2 output rows are written per core and concatenated on host.
"""

import math
from contextlib import ExitStack

import numpy as np

import concourse.bacc as bacc
import concourse.bass as bass
import concourse.tile as tile
from concourse import library_config, mybir
from concourse.bass_utils import run_bass_kernel_spmd

P = 128
D = 128
CORES = 8
N_NODES = 50000
N_EDGES = 800000

AF = mybir.ActivationFunctionType
OP = mybir.AluOpType
dt = mybir.dt


def _plan(n_nodes, cores):
    nloc = n_nodes // cores
    assert nloc * cores == n_nodes
    nb = math.ceil(nloc / P)
    nloc_pad = nb * P
    npad = cores * nloc_pad
    nhalf = npad // 2
    assert nhalf < 32768, "dma_gather idx is int16"
    return nloc, nb, nloc_pad, npad, nhalf


def preprocess(edge_index, n_nodes, cores):
    """Returns per-core gather/scatter metadata + uniform chunk counts K0, K1."""
    nloc, nb, nloc_pad, npad, nhalf = _plan(n_nodes, cores)
    src = np.asarray(edge_index[0], dtype=np.int64)
    dst = np.asarray(edge_index[1], dtype=np.int64)
    E = src.shape[0]

    deg = np.bincount(dst, minlength=n_nodes).astype(np.float64)
    invdeg = (1.0 / np.maximum(deg, 1.0)).astype(np.float32)

    csrc = src // nloc
    r_src = csrc * nloc_pad + (src - csrc * nloc)  # padded row id of source
    half = (r_src >= nhalf).astype(np.int64)
    idx_in_half = (r_src - half * nhalf).astype(np.int64)

    cdst = dst // nloc
    ldst = dst - cdst * nloc
    blk = ldst // P
    dblk = ldst % P

    # sort edges by (dst core, dst block, src half, src row) — src order gives
    # the DMA engines ascending-address locality within each gather list
    order = np.lexsort((idx_in_half, half, blk, cdst))
    s_half = half[order]
    s_idx = idx_in_half[order]
    s_dblk = dblk[order]
    key = ((cdst[order] * nb + blk[order]) * 2 + s_half).astype(np.int64)

    counts = np.bincount(key, minlength=cores * nb * 2)
    starts = np.zeros(cores * nb * 2 + 1, dtype=np.int64)
    np.cumsum(counts, out=starts[1:])
    rank = np.arange(E, dtype=np.int64) - starts[key]

    cnt = counts.reshape(cores, nb, 2)
    K0 = max(1, int(math.ceil(cnt[:, :, 0].max() / P)))
    K1 = max(1, int(math.ceil(cnt[:, :, 1].max() / P)))

    # idx arrays: [cores, nb, K*P] int16 (pad = 0, harmless row gathered,
    # neutralized by dloc pad = 255 in the one-hot); dloc: [cores, nb, (K0+K1)*P]
    fill = 0
    idx0 = np.full((cores, nb, K0 * P), fill, dtype=np.int16)
    idx1 = np.full((cores, nb, K1 * P), fill, dtype=np.int16)
    dloc = np.full((cores, nb, (K0 + K1) * P), 255.0, dtype=np.float32)

    core_k = key // (nb * 2)
    blk_k = (key // 2) % nb
    m0 = s_half == 0
    m1 = ~m0
    idx0[core_k[m0], blk_k[m0], rank[m0]] = s_idx[m0].astype(np.int16)
    idx1[core_k[m1], blk_k[m1], rank[m1]] = s_idx[m1].astype(np.int16)
    dloc[core_k[m0], blk_k[m0], rank[m0]] = s_dblk[m0].astype(np.float32)
    dloc[core_k[m1], blk_k[m1], K0 * P + rank[m1]] = s_dblk[m1].astype(np.float32)

    def wrap_idx(a):  # [nb, K*P] -> [128, nb*K*P//16] dma_gather layout
        flat = a.reshape(-1)
        w = flat.reshape(-1, 16).T  # [16, I/16]
        return np.tile(w, (8, 1)).copy()

    per_core = []
    for c in range(cores):
        dl = dloc[c].reshape(nb, K0 + K1, P).transpose(2, 0, 1).reshape(P, -1)
        inv = np.ones(nloc_pad, dtype=np.float32)
        inv[:nloc] = invdeg[c * nloc : (c + 1) * nloc]
        per_core.append(
            dict(
                idx0=wrap_idx(idx0[c]),
                idx1=wrap_idx(idx1[c]),
                dloc=np.ascontiguousarray(dl),
                invd=np.broadcast_to(inv[None, :], (P, nloc_pad)).copy(),
            )
        )
    return per_core, K0, K1, invdeg


def build_nc(n_nodes, cores, K0, K1, G, iters=1):
    import os
    abl = set(os.environ.get("BASS_ABL", "").split(","))
    SP = False       # single_packet=True hangs this runtime ucode
    NQ = 4           # spread gathers over all 4 SWDGE queue pairs
    FINE = int(os.environ.get("BASS_FINE", "1"))  # gather calls per (group, half)
    STB = 2
    PADNEG = False   # trailing -1 idx corrupts results on this ucode
    TABLOCAL = False
    OHPAIR = False   # one-hots via tensor_scalar (4x DVE mode) instead
    if PADNEG:
        assert FINE == 7, "trailing -1 pads require one call per (block, half)"
    nloc, nb, nloc_pad, npad, nhalf = _plan(n_nodes, cores)
    assert nb % G == 0
    ngroups = nb // G
    KT = K0 + K1

    nc = bacc.Bacc("TRN2", target_bir_lowering=False, debug=False, num_devices=cores, num_swdge_queues=NQ)

    xT_d = nc.dram_tensor("xT", [P, nloc_pad], dt.float16, kind="ExternalInput").ap()
    idx0_d = nc.dram_tensor("idx0", [P, nb * K0 * P // 16], dt.int16, kind="ExternalInput").ap()
    idx1_d = nc.dram_tensor("idx1", [P, nb * K1 * P // 16], dt.int16, kind="ExternalInput").ap()
    dloc_d = nc.dram_tensor("dloc", [P, nb * KT], dt.float32, kind="ExternalInput").ap()
    invd_d = nc.dram_tensor("invd", [P, nloc_pad], dt.float32, kind="ExternalInput").ap()
    wdram = {
        n: nc.dram_tensor(n, [P, D], dt.float16, kind="ExternalInput").ap()
        for n in ["Wp1T", "Wl1T", "Wr1T", "Wp2T", "Wl2T", "Wr2T"]
    }
    ones1_d = nc.dram_tensor("ones1", [1, P], dt.float16, kind="ExternalInput").ap()
    bp1r_d = nc.dram_tensor("bp1r", [1, D], dt.float16, kind="ExternalInput").ap()
    bp2r_d = nc.dram_tensor("bp2r", [1, D], dt.float16, kind="ExternalInput").ap()
    bl1c_d = nc.dram_tensor("bl1c", [P, 1], dt.float32, kind="ExternalInput").ap()
    bl2r_d = nc.dram_tensor("bl2r", [1, D], dt.float16, kind="ExternalInput").ap()
    iota_d = nc.dram_tensor("iota", [P, P], dt.float16, kind="ExternalInput").ap()

    out_own = nc.dram_tensor("out_own", [nloc_pad, D], dt.float32, kind="ExternalOutput").ap()
    h1own = nc.dram_tensor("h1own", [nloc_pad, D], dt.float16).ap()
    h2own = nc.dram_tensor("h2own", [nloc_pad, D], dt.float16).ap()
    tspace = "Local" if TABLOCAL else "Shared"
    table1 = nc.dram_tensor("table1", [npad, D], dt.float16, addr_space=tspace).ap()
    table2 = nc.dram_tensor("table2", [npad, D], dt.float16, addr_space=tspace).ap()

    groups_all = [list(range(cores))]

    with tile.TileContext(nc) as tc, ExitStack() as ctx:
        const = ctx.enter_context(tc.tile_pool(name="const", bufs=1))
        persist = ctx.enter_context(tc.tile_pool(name="persist", bufs=1))
        stage_p = ctx.enter_context(tc.tile_pool(name="stage", bufs=STB))
        work = ctx.enter_context(tc.tile_pool(name="work", bufs=3))
        ohp = ctx.enter_context(tc.tile_pool(name="oh", bufs=24))
        aggsb = ctx.enter_context(tc.tile_pool(name="aggsb", bufs=2))
        outp = ctx.enter_context(tc.tile_pool(name="outp", bufs=3))
        psum_agg = ctx.enter_context(tc.tile_pool(name="psum_agg", bufs=4, space="PSUM"))
        psum_mm = ctx.enter_context(tc.tile_pool(name="psum_mm", bufs=2, space="PSUM"))

        nc.gpsimd.load_library(library_config.mlp)

        def cload(ap_dram, shape, dtype, tag):
            t = const.tile(shape, dtype, tag=tag)
            nc.sync.dma_start(t[:], ap_dram)
            return t

        wsb = {n: cload(wdram[n][:, :], [P, D], dt.float16, n) for n in wdram}
        ones1 = cload(ones1_d[:, :], [1, P], dt.float16, "ones1")
        bp1r = cload(bp1r_d[:, :], [1, D], dt.float16, "bp1r")
        bp2r = cload(bp2r_d[:, :], [1, D], dt.float16, "bp2r")
        bl1c = cload(bl1c_d[:, :], [P, 1], dt.float32, "bl1c")
        bl2r = cload(bl2r_d[:, :], [1, D], dt.float16, "bl2r")
        iota = cload(iota_d[:, :], [P, P], dt.float16, "iota")
        dloc_sb = cload(dloc_d[:, :], [P, nb * KT], dt.float32, "dloc")
        invd_sb = cload(invd_d[:, :], [P, nloc_pad], dt.float32, "invd")
        idx0_sb = cload(idx0_d[:, :], [P, nb * K0 * P // 16], dt.int16, "idx0")
        idx1_sb = cload(idx1_d[:, :], [P, nb * K1 * P // 16], dt.int16, "idx1")

        xT_sb = cload(xT_d[:, :], [P, nloc_pad], dt.float16, "xT")
        h1T_sb = persist.tile([P, nloc_pad], dt.float16, tag="h1T")

        _abl_st = None
        if "nogather" in abl and "noagg" not in abl:
            _ast0 = persist.tile([P, G * K0, D], dt.float16, tag="ast0")
            _ast1 = persist.tile([P, G * K1, D], dt.float16, tag="ast1")
            _m0 = nc.vector.memset(_ast0[:], 0.25)
            _m1 = nc.vector.memset(_ast1[:], 0.25)
            _abl_st = (_ast0, _ast1)

        if PADNEG:
            for _i in range(STB):
                _pst0 = stage_p.tile([P, G * K0, D], dt.float16, tag="st0")
                _pm0 = nc.vector.memset(_pst0[:], 0.25)
                _pst1 = stage_p.tile([P, G * K1, D], dt.float16, tag="st1")
                _pm1 = nc.vector.memset(_pst1[:], 0.25)

        def _iter_body():
            # ---------------- Phase A: layer-1 projection of own rows ----------
            for b in range(nb):
                sl = slice(b * P, (b + 1) * P)
                p_ps = psum_mm.tile([P, D], dt.float32, tag="mm")
                nc.tensor.matmul(p_ps[:], lhsT=ones1[:], rhs=bp1r[:], start=True, stop=False)
                nc.tensor.matmul(p_ps[:], lhsT=xT_sb[:, sl], rhs=wsb["Wp1T"][:], start=False, stop=True)
                pr = outp.tile([P, D], dt.float16, tag="pr")
                nc.scalar.activation(pr[:], p_ps[:], AF.Relu)
                nc.sync.dma_start(h1own[sl, :], pr[:])

            if "nocc" not in abl:
                nc.gpsimd.collective_compute(
                    "AllGather", OP.bypass, replica_groups=groups_all,
                    ins=[h1own[:, :]], outs=[table1[:, :]],
                )

            # ---------------- message+aggregate for one layer -------------------
            def agg_layer(table, root_sb, WlT, WrT, layer):
                for g in range(ngroups):
                    if "noagg" in abl and "nogather" in abl:
                        continue
                    if "nogather" in abl:
                        st0 = _abl_st[0]
                        st1 = _abl_st[1]
                    else:
                        st0 = stage_p.tile([P, G * K0, D], dt.float16, tag="st0")
                        st1 = stage_p.tile([P, G * K1, D], dt.float16, tag="st1")
                        c0 = G * K0 * P // 16
                        c1 = G * K1 * P // 16
                        assert (G * K0) % FINE == 0 and (G * K1) % FINE == 0
                        for f in range(FINE):
                            f0 = c0 // FINE
                            nc.gpsimd.dma_gather(
                                st0[:, f * (G * K0) // FINE : (f + 1) * (G * K0) // FINE, :],
                                table[0:nhalf, :],
                                idx0_sb[:, g * c0 + f * f0 : g * c0 + (f + 1) * f0],
                                G * K0 * P // FINE, G * K0 * P // FINE, D, single_packet=SP,
                                queue_num=(2 * g * FINE + 2 * f) % NQ,
                            )
                            f1 = c1 // FINE
                            nc.gpsimd.dma_gather(
                                st1[:, f * (G * K1) // FINE : (f + 1) * (G * K1) // FINE, :],
                                table[nhalf:npad, :],
                                idx1_sb[:, g * c1 + f * f1 : g * c1 + (f + 1) * f1],
                                G * K1 * P // FINE, G * K1 * P // FINE, D, single_packet=SP,
                                queue_num=(2 * g * FINE + 2 * f + 1) % NQ,
                            )
                    if "noagg" in abl:
                        continue
                    for bb in range(G):
                        b = g * G + bb
                        sl = slice(b * P, (b + 1) * P)
                        agg_ps = psum_agg.tile([P, P], dt.float32)
                        if OHPAIR:
                            ohs = []
                            for t0 in range(0, KT, 2):
                                npair = min(2, KT - t0)
                                ohw = ohp.tile([P, 2, P], dt.float16, tag="ohw")
                                nc.vector.tensor_tensor(
                                    out=ohw[:, 0:npair, :],
                                    in0=dloc_sb[:, b * KT + t0 : b * KT + t0 + npair]
                                    .unsqueeze(2)
                                    .to_broadcast([P, npair, P]),
                                    in1=iota[:].unsqueeze(1).to_broadcast([P, npair, P]),
                                    op=OP.is_equal,
                                )
                                ohs.append(ohw)
                        for t in range(KT):
                            if OHPAIR:
                                oh = ohs[t // 2][:, t % 2, :]
                            else:
                                # tensor_scalar is_equal vs a [P,1] fp32 scalar
                                # keeps the DVE 4x perf mode (packed fp16 in/out)
                                oh_t = ohp.tile([P, P], dt.float16, tag="ohw")
                                nc.vector.tensor_scalar(
                                    out=oh_t[:],
                                    in0=iota[:],
                                    scalar1=dloc_sb[:, b * KT + t : b * KT + t + 1],
                                    scalar2=None,
                                    op0=OP.is_equal,
                                )
                                oh = oh_t[:]
                            msg = st0[:, bb * K0 + t, :] if t < K0 else st1[:, bb * K1 + (t - K0), :]
                            nc.tensor.matmul(
                                agg_ps[:], lhsT=msg, rhs=oh,
                                start=(t == 0), stop=(t == KT - 1),
                            )
                        aggT = aggsb.tile([P, P], dt.float16)
                        nc.vector.tensor_tensor(
                            out=aggT[:], in0=agg_ps[:], in1=invd_sb[:, sl], op=OP.mult
                        )
                        if layer == 1:
                            o_ps = psum_mm.tile([P, P], dt.float32, tag="mm")
                            nc.tensor.matmul(o_ps[:], lhsT=WlT[:], rhs=aggT[:], start=True, stop=False)
                            nc.tensor.matmul(o_ps[:], lhsT=WrT[:], rhs=root_sb[:, sl], start=False, stop=True)
                            nc.scalar.activation(h1T_sb[:, sl], o_ps[:], AF.Relu, bias=bl1c[:], scale=1.0)
                            # fused layer-2 projection of this block
                            ps2 = psum_mm.tile([P, D], dt.float32, tag="mm")
                            nc.tensor.matmul(ps2[:], lhsT=ones1[:], rhs=bp2r[:], start=True, stop=False)
                            nc.tensor.matmul(ps2[:], lhsT=h1T_sb[:, sl], rhs=wsb["Wp2T"][:], start=False, stop=True)
                            pr2 = outp.tile([P, D], dt.float16, tag="pr")
                            nc.scalar.activation(pr2[:], ps2[:], AF.Relu)
                            nc.sync.dma_start(h2own[sl, :], pr2[:])
                        else:
                            o_ps = psum_mm.tile([P, D], dt.float32, tag="mm")
                            nc.tensor.matmul(o_ps[:], lhsT=ones1[:], rhs=bl2r[:], start=True, stop=False)
                            nc.tensor.matmul(o_ps[:], lhsT=aggT[:], rhs=WlT[:], start=False, stop=False)
                            nc.tensor.matmul(o_ps[:], lhsT=root_sb[:, sl], rhs=WrT[:], start=False, stop=True)
                            ob = outp.tile([P, D], dt.float32, tag="ob")
                            nc.scalar.activation(ob[:], o_ps[:], AF.Copy)
                            nc.sync.dma_start(out_own[sl, :], ob[:])

            # ---------------- Phase B: layer-1 aggregate -> h1T -----------------
            agg_layer(table1, xT_sb, wsb["Wl1T"], wsb["Wr1T"], layer=1)

            if "nocc" not in abl:
                nc.gpsimd.collective_compute(
                    "AllGather", OP.bypass, replica_groups=groups_all,
                    ins=[h2own[:, :]], outs=[table2[:, :]],
                )

            # ---------------- Phase D: layer-2 aggregate -> out ----------------
            agg_layer(table2, xT_sb if "noagg" in abl else h1T_sb, wsb["Wl2T"], wsb["Wr2T"], layer=2)

        for _ in range(iters):
            _iter_body()

    nc.compile()
    return nc


def make_in_maps(inputs, per_core, n_nodes, cores):
    nloc, nb, nloc_pad, npad, nhalf = _plan(n_nodes, cores)
    x = np.asarray(inputs["x"], dtype=np.float32)
    consts = dict(
        Wp1T=np.asarray(inputs["Wp1"]).T.astype(np.float16),
        Wl1T=np.asarray(inputs["Wl1"]).T.astype(np.float16),
        Wr1T=np.asarray(inputs["Wr1"]).T.astype(np.float16),
        Wp2T=np.asarray(inputs["Wp2"]).T.astype(np.float16),
        Wl2T=np.asarray(inputs["Wl2"]).T.astype(np.float16),
        Wr2T=np.asarray(inputs["Wr2"]).T.astype(np.float16),
        ones1=np.ones((1, P), dtype=np.float16),
        bp1r=np.asarray(inputs["bp1"], np.float32).reshape(1, D).astype(np.float16),
        bp2r=np.asarray(inputs["bp2"], np.float32).reshape(1, D).astype(np.float16),
        bl1c=np.asarray(inputs["bl1"], np.float32).reshape(P, 1).copy(),
        bl2r=np.asarray(inputs["bl2"], np.float32).reshape(1, D).astype(np.float16),
        iota=np.broadcast_to(np.arange(P, dtype=np.float16)[None, :], (P, P)).copy(),
    )
    in_maps = []
    for c in range(cores):
        xo = np.zeros((nloc_pad, D), dtype=np.float32)
        xo[:nloc] = x[c * nloc : (c + 1) * nloc]
        m = dict(consts)
        m["xT"] = np.ascontiguousarray(xo.T).astype(np.float16)
        m.update(per_core[c])
        in_maps.append(m)
    return in_maps


_BUILT = {}


def _run(inputs, n_nodes, n_edges, cores, G, trace=False):
    per_core, K0, K1, _ = preprocess(inputs["edge_index"], n_nodes, cores)
    key = (n_nodes, cores, K0, K1, G)
    if key not in _BUILT:
        _BUILT[key] = build_nc(n_nodes, cores, K0, K1, G)
    nc = _BUILT[key]
    in_maps = make_in_maps(inputs, per_core, n_nodes, cores)
    res = run_bass_kernel_spmd(nc, in_maps, list(range(cores)), trace=trace)
    nloc, nb, nloc_pad, npad, nhalf = _plan(n_nodes, cores)
    out = np.concatenate([res.results[c]["out_own"][:nloc] for c in range(cores)], axis=0)
    return out.astype(np.float32), res


def kernel(**inputs):
    out, _ = _run(inputs, N_NODES, N_EDGES, CORES, G=7)
    return out



# revision 28
# speedup vs baseline: 2.3193x; 2.3193x over previous
"""2-layer GraphSAGE on 8 trn2 NeuronCores - v4.

v3 dataflow (HBM-table dma_gather + one-hot scatter matmuls, 4 SWDGE queues,
pre-transposed fp16 xT, bias-fold matmuls, fused layer-2 projection) plus:
  - one-hots built in a few large broadcast is_equal ops per block (OHG=6,
    3 DVE ops/block): HW per-instruction overhead (~250ns) dominates, so
    fewer, larger DVE ops beat many small ones,
  - gather split into 9 calls per (group, half) (FINE=9),
  - psum_mm triple-buffered for cross-phase overlap.

Strategy (graph/data parallel, hardcoded for N=50000, E=800000, D=128, 8 cores):
  - Nodes sharded by contiguous ranges of 6250 (padded to 6272 = 49*128) per core.
  - Host preprocesses edges: sorted by (dst core, dst block, src half, src),
    padded so every (block, half) has a uniform chunk count across cores (SPMD).
  - Device per layer:
      * project own rows: p = relu(x @ WpT + bp)  -> fp16, AllGather into a
        replicated [50176,128] fp16 table in DRAM.
      * dma_gather (SWDGE) message rows from the table (two int16-indexed
        halves), 128 edges per chunk.
      * scatter via one-hot matmuls: aggT[k,d] += msg[e,k]^T @ onehot[e,d],
        onehot built on DVE with is_equal against an iota tile.
      * mean via per-dst invdeg multiply, then output matmuls + bias (+relu).
  - Layer-2 output rows are written per core and concatenated on host.
"""

import math
from contextlib import ExitStack

import numpy as np

import concourse.bacc as bacc
import concourse.bass as bass
import concourse.tile as tile
from concourse import library_config, mybir
from concourse.bass_utils import run_bass_kernel_spmd

P = 128
D = 128
CORES = 8
N_NODES = 50000
N_EDGES = 800000

AF = mybir.ActivationFunctionType
OP = mybir.AluOpType
dt = mybir.dt


def _plan(n_nodes, cores):
    nloc = n_nodes // cores
    assert nloc * cores == n_nodes
    nb = math.ceil(nloc / P)
    nloc_pad = nb * P
    npad = cores * nloc_pad
    nhalf = npad // 2
    assert nhalf < 32768, "dma_gather idx is int16"
    return nloc, nb, nloc_pad, npad, nhalf


def preprocess(edge_index, n_nodes, cores):
    """Returns per-core gather/scatter metadata + uniform chunk counts K0, K1."""
    nloc, nb, nloc_pad, npad, nhalf = _plan(n_nodes, cores)
    src = np.asarray(edge_index[0], dtype=np.int64)
    dst = np.asarray(edge_index[1], dtype=np.int64)
    E = src.shape[0]

    deg = np.bincount(dst, minlength=n_nodes).astype(np.float64)
    invdeg = (1.0 / np.maximum(deg, 1.0)).astype(np.float32)

    csrc = src // nloc
    r_src = csrc * nloc_pad + (src - csrc * nloc)  # padded row id of source
    half = (r_src >= nhalf).astype(np.int64)
    idx_in_half = (r_src - half * nhalf).astype(np.int64)

    cdst = dst // nloc
    ldst = dst - cdst * nloc
    blk = ldst // P
    dblk = ldst % P

    # sort edges by (dst core, dst block, src half, src row) — src order gives
    # the DMA engines ascending-address locality within each gather list
    order = np.lexsort((idx_in_half, half, blk, cdst))
    s_half = half[order]
    s_idx = idx_in_half[order]
    s_dblk = dblk[order]
    key = ((cdst[order] * nb + blk[order]) * 2 + s_half).astype(np.int64)

    counts = np.bincount(key, minlength=cores * nb * 2)
    starts = np.zeros(cores * nb * 2 + 1, dtype=np.int64)
    np.cumsum(counts, out=starts[1:])
    rank = np.arange(E, dtype=np.int64) - starts[key]

    cnt = counts.reshape(cores, nb, 2)
    K0 = max(1, int(math.ceil(cnt[:, :, 0].max() / P)))
    K1 = max(1, int(math.ceil(cnt[:, :, 1].max() / P)))

    # idx arrays: [cores, nb, K*P] int16 (pad = 0, harmless row gathered,
    # neutralized by dloc pad = 255 in the one-hot); dloc: [cores, nb, (K0+K1)*P]
    fill = 0
    idx0 = np.full((cores, nb, K0 * P), fill, dtype=np.int16)
    idx1 = np.full((cores, nb, K1 * P), fill, dtype=np.int16)
    dloc = np.full((cores, nb, (K0 + K1) * P), 255.0, dtype=np.float16)

    core_k = key // (nb * 2)
    blk_k = (key // 2) % nb
    m0 = s_half == 0
    m1 = ~m0
    idx0[core_k[m0], blk_k[m0], rank[m0]] = s_idx[m0].astype(np.int16)
    idx1[core_k[m1], blk_k[m1], rank[m1]] = s_idx[m1].astype(np.int16)
    dloc[core_k[m0], blk_k[m0], rank[m0]] = s_dblk[m0].astype(np.float16)
    dloc[core_k[m1], blk_k[m1], K0 * P + rank[m1]] = s_dblk[m1].astype(np.float16)

    def wrap_idx(a):  # [nb, K*P] -> [128, nb*K*P//16] dma_gather layout
        flat = a.reshape(-1)
        w = flat.reshape(-1, 16).T  # [16, I/16]
        return np.tile(w, (8, 1)).copy()

    per_core = []
    for c in range(cores):
        dl = dloc[c].reshape(nb, K0 + K1, P).transpose(2, 0, 1).reshape(P, -1)
        inv = np.ones(nloc_pad, dtype=np.float32)
        inv[:nloc] = invdeg[c * nloc : (c + 1) * nloc]
        per_core.append(
            dict(
                idx0=wrap_idx(idx0[c]),
                idx1=wrap_idx(idx1[c]),
                dloc=np.ascontiguousarray(dl),
                invd=np.broadcast_to(inv[None, :], (P, nloc_pad)).copy(),
            )
        )
    return per_core, K0, K1, invdeg


def build_nc(n_nodes, cores, K0, K1, G, iters=1):
    import os
    abl = set(os.environ.get("BASS_ABL", "").split(","))
    SP = False       # single_packet=True hangs this runtime ucode (re-confirmed)
    NQ = 4           # spread gathers over all 4 SWDGE queue pairs
    FINE = int(os.environ.get("BASS_FINE", "9"))  # gather calls per (group, half)
    STB = int(os.environ.get("BASS_STB", "2"))
    PADNEG = False   # trailing -1 idx corrupts results on this ucode
    TABLOCAL = False
    OHG = int(os.environ.get("BASS_OHG", "6"))  # one-hot chunks per DVE op (0 = block)
    QRR = os.environ.get("BASS_QRR", "0") == "1"  # global round-robin queue pick
    if PADNEG:
        assert FINE == 7, "trailing -1 pads require one call per (block, half)"
    nloc, nb, nloc_pad, npad, nhalf = _plan(n_nodes, cores)
    assert nb % G == 0
    ngroups = nb // G
    KT = K0 + K1

    nc = bacc.Bacc("TRN2", target_bir_lowering=False, debug=False, num_devices=cores, num_swdge_queues=NQ)

    xT_d = nc.dram_tensor("xT", [P, nloc_pad], dt.float16, kind="ExternalInput").ap()
    idx0_d = nc.dram_tensor("idx0", [P, nb * K0 * P // 16], dt.int16, kind="ExternalInput").ap()
    idx1_d = nc.dram_tensor("idx1", [P, nb * K1 * P // 16], dt.int16, kind="ExternalInput").ap()
    dloc_d = nc.dram_tensor("dloc", [P, nb * KT], dt.float16, kind="ExternalInput").ap()
    invd_d = nc.dram_tensor("invd", [P, nloc_pad], dt.float32, kind="ExternalInput").ap()
    wdram = {
        n: nc.dram_tensor(n, [P, D], dt.float16, kind="ExternalInput").ap()
        for n in ["Wp1T", "Wl1T", "Wr1T", "Wp2T", "Wl2T", "Wr2T"]
    }
    ones1_d = nc.dram_tensor("ones1", [1, P], dt.float16, kind="ExternalInput").ap()
    bp1r_d = nc.dram_tensor("bp1r", [1, D], dt.float16, kind="ExternalInput").ap()
    bp2r_d = nc.dram_tensor("bp2r", [1, D], dt.float16, kind="ExternalInput").ap()
    bl1c_d = nc.dram_tensor("bl1c", [P, 1], dt.float32, kind="ExternalInput").ap()
    bl2r_d = nc.dram_tensor("bl2r", [1, D], dt.float16, kind="ExternalInput").ap()
    iota_d = nc.dram_tensor("iota", [P, P], dt.float16, kind="ExternalInput").ap()

    out_own = nc.dram_tensor("out_own", [nloc_pad, D], dt.float32, kind="ExternalOutput").ap()
    h1own = nc.dram_tensor("h1own", [nloc_pad, D], dt.float16).ap()
    h2own = nc.dram_tensor("h2own", [nloc_pad, D], dt.float16).ap()
    tspace = "Local" if TABLOCAL else "Shared"
    table1 = nc.dram_tensor("table1", [npad, D], dt.float16, addr_space=tspace).ap()
    table2 = nc.dram_tensor("table2", [npad, D], dt.float16, addr_space=tspace).ap()

    groups_all = [list(range(cores))]

    with tile.TileContext(nc) as tc, ExitStack() as ctx:
        const = ctx.enter_context(tc.tile_pool(name="const", bufs=1))
        persist = ctx.enter_context(tc.tile_pool(name="persist", bufs=1))
        stage_p = ctx.enter_context(tc.tile_pool(name="stage", bufs=STB))
        work = ctx.enter_context(tc.tile_pool(name="work", bufs=3))
        _ohg = KT if OHG == 0 else OHG
        ohp = ctx.enter_context(
            tc.tile_pool(name="oh", bufs=3 * math.ceil(KT / _ohg))
        )
        aggsb = ctx.enter_context(tc.tile_pool(name="aggsb", bufs=2))
        outp = ctx.enter_context(tc.tile_pool(name="outp", bufs=3))
        psum_agg = ctx.enter_context(tc.tile_pool(name="psum_agg", bufs=4, space="PSUM"))
        psum_mm = ctx.enter_context(tc.tile_pool(name="psum_mm", bufs=3, space="PSUM"))

        nc.gpsimd.load_library(library_config.mlp)

        def cload(ap_dram, shape, dtype, tag):
            t = const.tile(shape, dtype, tag=tag)
            nc.sync.dma_start(t[:], ap_dram)
            return t

        wsb = {n: cload(wdram[n][:, :], [P, D], dt.float16, n) for n in wdram}
        ones1 = cload(ones1_d[:, :], [1, P], dt.float16, "ones1")
        bp1r = cload(bp1r_d[:, :], [1, D], dt.float16, "bp1r")
        bp2r = cload(bp2r_d[:, :], [1, D], dt.float16, "bp2r")
        bl1c = cload(bl1c_d[:, :], [P, 1], dt.float32, "bl1c")
        bl2r = cload(bl2r_d[:, :], [1, D], dt.float16, "bl2r")
        iota = cload(iota_d[:, :], [P, P], dt.float16, "iota")
        dloc_sb = cload(dloc_d[:, :], [P, nb * KT], dt.float16, "dloc")
        invd_sb = cload(invd_d[:, :], [P, nloc_pad], dt.float32, "invd")
        idx0_sb = cload(idx0_d[:, :], [P, nb * K0 * P // 16], dt.int16, "idx0")
        idx1_sb = cload(idx1_d[:, :], [P, nb * K1 * P // 16], dt.int16, "idx1")

        xT_sb = cload(xT_d[:, :], [P, nloc_pad], dt.float16, "xT")
        h1T_sb = persist.tile([P, nloc_pad], dt.float16, tag="h1T")

        _abl_st = None
        if "nogather" in abl and "noagg" not in abl:
            _ast0 = persist.tile([P, G * K0, D], dt.float16, tag="ast0")
            _ast1 = persist.tile([P, G * K1, D], dt.float16, tag="ast1")
            _m0 = nc.vector.memset(_ast0[:], 0.25)
            _m1 = nc.vector.memset(_ast1[:], 0.25)
            _abl_st = (_ast0, _ast1)

        if PADNEG:
            for _i in range(STB):
                _pst0 = stage_p.tile([P, G * K0, D], dt.float16, tag="st0")
                _pm0 = nc.vector.memset(_pst0[:], 0.25)
                _pst1 = stage_p.tile([P, G * K1, D], dt.float16, tag="st1")
                _pm1 = nc.vector.memset(_pst1[:], 0.25)

        _qctr = [0]

        def _next_q(default):
            if not QRR:
                return default
            q = _qctr[0] % NQ
            _qctr[0] += 1
            return q

        def _iter_body():
            # ---------------- Phase A: layer-1 projection of own rows ----------
            for b in range(nb):
                sl = slice(b * P, (b + 1) * P)
                p_ps = psum_mm.tile([P, D], dt.float32, tag="mm")
                nc.tensor.matmul(p_ps[:], lhsT=ones1[:], rhs=bp1r[:], start=True, stop=False)
                nc.tensor.matmul(p_ps[:], lhsT=xT_sb[:, sl], rhs=wsb["Wp1T"][:], start=False, stop=True)
                pr = outp.tile([P, D], dt.float16, tag="pr")
                nc.scalar.activation(pr[:], p_ps[:], AF.Relu)
                nc.sync.dma_start(h1own[sl, :], pr[:])

            if "nocc" not in abl:
                nc.gpsimd.collective_compute(
                    "AllGather", OP.bypass, replica_groups=groups_all,
                    ins=[h1own[:, :]], outs=[table1[:, :]],
                )

            # ---------------- message+aggregate for one layer -------------------
            def agg_layer(table, root_sb, WlT, WrT, layer):
                for g in range(ngroups):
                    if "noagg" in abl and "nogather" in abl:
                        continue
                    if "nogather" in abl:
                        st0 = _abl_st[0]
                        st1 = _abl_st[1]
                    else:
                        st0 = stage_p.tile([P, G * K0, D], dt.float16, tag="st0")
                        st1 = stage_p.tile([P, G * K1, D], dt.float16, tag="st1")
                        c0 = G * K0 * P // 16
                        c1 = G * K1 * P // 16
                        assert (G * K0) % FINE == 0 and (G * K1) % FINE == 0
                        for f in range(FINE):
                            f0 = c0 // FINE
                            nc.gpsimd.dma_gather(
                                st0[:, f * (G * K0) // FINE : (f + 1) * (G * K0) // FINE, :],
                                table[0:nhalf, :],
                                idx0_sb[:, g * c0 + f * f0 : g * c0 + (f + 1) * f0],
                                G * K0 * P // FINE, G * K0 * P // FINE, D, single_packet=SP,
                                queue_num=_next_q((2 * g * FINE + 2 * f) % NQ),
                            )
                            f1 = c1 // FINE
                            nc.gpsimd.dma_gather(
                                st1[:, f * (G * K1) // FINE : (f + 1) * (G * K1) // FINE, :],
                                table[nhalf:npad, :],
                                idx1_sb[:, g * c1 + f * f1 : g * c1 + (f + 1) * f1],
                                G * K1 * P // FINE, G * K1 * P // FINE, D, single_packet=SP,
                                queue_num=_next_q((2 * g * FINE + 2 * f + 1) % NQ),
                            )
                    if "noagg" in abl:
                        continue
                    for bb in range(G):
                        b = g * G + bb
                        sl = slice(b * P, (b + 1) * P)
                        agg_ps = psum_agg.tile([P, P], dt.float32)
                        # one-hots in few large DVE ops per block: HW per-op
                        # overhead (~250ns) dominates, so amortize it
                        ohg = KT if OHG == 0 else OHG
                        ohs = []
                        for t0 in range(0, KT, ohg):
                            nh = min(ohg, KT - t0)
                            ohw = ohp.tile([P, ohg, P], dt.float16, tag="ohw")
                            nc.vector.tensor_tensor(
                                out=ohw[:, 0:nh, :],
                                in0=dloc_sb[:, b * KT + t0 : b * KT + t0 + nh]
                                .unsqueeze(2)
                                .to_broadcast([P, nh, P]),
                                in1=iota[:].unsqueeze(1).to_broadcast([P, nh, P]),
                                op=OP.is_equal,
                            )
                            ohs.append(ohw)
                        for t in range(KT):
                            oh = ohs[t // ohg][:, t % ohg, :]
                            msg = st0[:, bb * K0 + t, :] if t < K0 else st1[:, bb * K1 + (t - K0), :]
                            nc.tensor.matmul(
                                agg_ps[:], lhsT=msg, rhs=oh,
                                start=(t == 0), stop=(t == KT - 1),
                            )
                        aggT = aggsb.tile([P, P], dt.float16)
                        nc.vector.tensor_tensor(
                            out=aggT[:], in0=agg_ps[:], in1=invd_sb[:, sl], op=OP.mult
                        )
                        if layer == 1:
                            o_ps = psum_mm.tile([P, P], dt.float32, tag="mm")
                            nc.tensor.matmul(o_ps[:], lhsT=WlT[:], rhs=aggT[:], start=True, stop=False)
                            nc.tensor.matmul(o_ps[:], lhsT=WrT[:], rhs=root_sb[:, sl], start=False, stop=True)
                            nc.scalar.activation(h1T_sb[:, sl], o_ps[:], AF.Relu, bias=bl1c[:], scale=1.0)
                            # fused layer-2 projection of this block
                            ps2 = psum_mm.tile([P, D], dt.float32, tag="mm")
                            nc.tensor.matmul(ps2[:], lhsT=ones1[:], rhs=bp2r[:], start=True, stop=False)
                            nc.tensor.matmul(ps2[:], lhsT=h1T_sb[:, sl], rhs=wsb["Wp2T"][:], start=False, stop=True)
                            pr2 = outp.tile([P, D], dt.float16, tag="pr")
                            nc.scalar.activation(pr2[:], ps2[:], AF.Relu)
                            nc.sync.dma_start(h2own[sl, :], pr2[:])
                        else:
                            o_ps = psum_mm.tile([P, D], dt.float32, tag="mm")
                            nc.tensor.matmul(o_ps[:], lhsT=ones1[:], rhs=bl2r[:], start=True, stop=False)
                            nc.tensor.matmul(o_ps[:], lhsT=aggT[:], rhs=WlT[:], start=False, stop=False)
                            nc.tensor.matmul(o_ps[:], lhsT=root_sb[:, sl], rhs=WrT[:], start=False, stop=True)
                            ob = outp.tile([P, D], dt.float32, tag="ob")
                            nc.scalar.activation(ob[:], o_ps[:], AF.Copy)
                            nc.sync.dma_start(out_own[sl, :], ob[:])

            # ---------------- Phase B: layer-1 aggregate -> h1T -----------------
            agg_layer(table1, xT_sb, wsb["Wl1T"], wsb["Wr1T"], layer=1)

            if "nocc" not in abl:
                nc.gpsimd.collective_compute(
                    "AllGather", OP.bypass, replica_groups=groups_all,
                    ins=[h2own[:, :]], outs=[table2[:, :]],
                )

            # ---------------- Phase D: layer-2 aggregate -> out ----------------
            agg_layer(table2, xT_sb if "noagg" in abl else h1T_sb, wsb["Wl2T"], wsb["Wr2T"], layer=2)

        for _ in range(iters):
            _iter_body()

    nc.compile()
    return nc


def make_in_maps(inputs, per_core, n_nodes, cores):
    nloc, nb, nloc_pad, npad, nhalf = _plan(n_nodes, cores)
    x = np.asarray(inputs["x"], dtype=np.float32)
    consts = dict(
        Wp1T=np.asarray(inputs["Wp1"]).T.astype(np.float16),
        Wl1T=np.asarray(inputs["Wl1"]).T.astype(np.float16),
        Wr1T=np.asarray(inputs["Wr1"]).T.astype(np.float16),
        Wp2T=np.asarray(inputs["Wp2"]).T.astype(np.float16),
        Wl2T=np.asarray(inputs["Wl2"]).T.astype(np.float16),
        Wr2T=np.asarray(inputs["Wr2"]).T.astype(np.float16),
        ones1=np.ones((1, P), dtype=np.float16),
        bp1r=np.asarray(inputs["bp1"], np.float32).reshape(1, D).astype(np.float16),
        bp2r=np.asarray(inputs["bp2"], np.float32).reshape(1, D).astype(np.float16),
        bl1c=np.asarray(inputs["bl1"], np.float32).reshape(P, 1).copy(),
        bl2r=np.asarray(inputs["bl2"], np.float32).reshape(1, D).astype(np.float16),
        iota=np.broadcast_to(np.arange(P, dtype=np.float16)[None, :], (P, P)).copy(),
    )
    in_maps = []
    for c in range(cores):
        xo = np.zeros((nloc_pad, D), dtype=np.float32)
        xo[:nloc] = x[c * nloc : (c + 1) * nloc]
        m = dict(consts)
        m["xT"] = np.ascontiguousarray(xo.T).astype(np.float16)
        m.update(per_core[c])
        in_maps.append(m)
    return in_maps


_BUILT = {}


def _run(inputs, n_nodes, n_edges, cores, G, trace=False):
    per_core, K0, K1, _ = preprocess(inputs["edge_index"], n_nodes, cores)
    key = (n_nodes, cores, K0, K1, G)
    if key not in _BUILT:
        _BUILT[key] = build_nc(n_nodes, cores, K0, K1, G)
    nc = _BUILT[key]
    in_maps = make_in_maps(inputs, per_core, n_nodes, cores)
    res = run_bass_kernel_spmd(nc, in_maps, list(range(cores)), trace=trace)
    nloc, nb, nloc_pad, npad, nhalf = _plan(n_nodes, cores)
    out = np.concatenate([res.results[c]["out_own"][:nloc] for c in range(cores)], axis=0)
    return out.astype(np.float32), res


def kernel(**inputs):
    out, _ = _run(inputs, N_NODES, N_EDGES, CORES, G=7)
    return out



# revision 30
# speedup vs baseline: 2.6502x; 1.1427x over previous
"""2-layer GraphSAGE on 8 trn2 NeuronCores - v4.

v3 dataflow (HBM-table dma_gather + one-hot scatter matmuls, 4 SWDGE queues,
pre-transposed fp16 xT, bias-fold matmuls, fused layer-2 projection) plus:
  - one-hots built in a few large broadcast is_equal ops per block (OHG=6,
    3 DVE ops/block): HW per-instruction overhead (~250ns) dominates, so
    fewer, larger DVE ops beat many small ones,
  - gather split into 9 calls per (group, half) (FINE=9),
  - psum_mm triple-buffered for cross-phase overlap.

Strategy (graph/data parallel, hardcoded for N=50000, E=800000, D=128, 8 cores):
  - Nodes sharded by contiguous ranges of 6250 (padded to 6272 = 49*128) per core.
  - Host preprocesses edges: sorted by (dst core, dst block, src half, src),
    padded so every (block, half) has a uniform chunk count across cores (SPMD).
  - Device per layer:
      * project own rows: p = relu(x @ WpT + bp)  -> fp16, AllGather into a
        replicated [50176,128] fp16 table in DRAM.
      * dma_gather (SWDGE) message rows from the table (two int16-indexed
        halves), 128 edges per chunk.
      * scatter via one-hot matmuls: aggT[k,d] += msg[e,k]^T @ onehot[e,d],
        onehot built on DVE with is_equal against an iota tile.
      * mean via per-dst invdeg multiply, then output matmuls + bias (+relu).
  - Layer-2 output rows are written per core and concatenated on host.
"""

import math
from contextlib import ExitStack

import numpy as np

import concourse.bacc as bacc
import concourse.bass as bass
import concourse.tile as tile
from concourse import library_config, mybir
from concourse.bass_utils import run_bass_kernel_spmd

P = 128
D = 128
CORES = 8
N_NODES = 50000
N_EDGES = 800000

AF = mybir.ActivationFunctionType
OP = mybir.AluOpType
dt = mybir.dt


def _plan(n_nodes, cores):
    nloc = n_nodes // cores
    assert nloc * cores == n_nodes
    nb = math.ceil(nloc / P)
    nloc_pad = nb * P
    npad = cores * nloc_pad
    nhalf = npad // 2
    assert nhalf < 32768, "dma_gather idx is int16"
    return nloc, nb, nloc_pad, npad, nhalf


def preprocess(edge_index, n_nodes, cores):
    """Returns per-core gather/scatter metadata + uniform chunk counts K0, K1."""
    nloc, nb, nloc_pad, npad, nhalf = _plan(n_nodes, cores)
    src = np.asarray(edge_index[0], dtype=np.int64)
    dst = np.asarray(edge_index[1], dtype=np.int64)
    E = src.shape[0]

    deg = np.bincount(dst, minlength=n_nodes).astype(np.float64)
    invdeg = (1.0 / np.maximum(deg, 1.0)).astype(np.float32)

    csrc = src // nloc
    r_src = csrc * nloc_pad + (src - csrc * nloc)  # padded row id of source
    half = (r_src >= nhalf).astype(np.int64)
    idx_in_half = (r_src - half * nhalf).astype(np.int64)

    cdst = dst // nloc
    ldst = dst - cdst * nloc
    blk = ldst // P
    dblk = ldst % P

    # sort edges by (dst core, dst block, src half, src row) — src order gives
    # the DMA engines ascending-address locality within each gather list
    order = np.lexsort((idx_in_half, half, blk, cdst))
    s_half = half[order]
    s_idx = idx_in_half[order]
    s_dblk = dblk[order]
    key = ((cdst[order] * nb + blk[order]) * 2 + s_half).astype(np.int64)

    counts = np.bincount(key, minlength=cores * nb * 2)
    starts = np.zeros(cores * nb * 2 + 1, dtype=np.int64)
    np.cumsum(counts, out=starts[1:])
    rank = np.arange(E, dtype=np.int64) - starts[key]

    cnt = counts.reshape(cores, nb, 2)
    K0 = max(1, int(math.ceil(cnt[:, :, 0].max() / P)))
    K1 = max(1, int(math.ceil(cnt[:, :, 1].max() / P)))

    # idx arrays: [cores, nb, K*P] int16 (pad = 0, harmless row gathered,
    # neutralized by dloc pad = 255 in the one-hot); dloc: [cores, nb, (K0+K1)*P]
    fill = 0
    idx0 = np.full((cores, nb, K0 * P), fill, dtype=np.int16)
    idx1 = np.full((cores, nb, K1 * P), fill, dtype=np.int16)
    dloc = np.full((cores, nb, (K0 + K1) * P), 255.0, dtype=np.float16)

    core_k = key // (nb * 2)
    blk_k = (key // 2) % nb
    m0 = s_half == 0
    m1 = ~m0
    idx0[core_k[m0], blk_k[m0], rank[m0]] = s_idx[m0].astype(np.int16)
    idx1[core_k[m1], blk_k[m1], rank[m1]] = s_idx[m1].astype(np.int16)
    dloc[core_k[m0], blk_k[m0], rank[m0]] = s_dblk[m0].astype(np.float16)
    dloc[core_k[m1], blk_k[m1], K0 * P + rank[m1]] = s_dblk[m1].astype(np.float16)

    def wrap_idx(a):  # [nb, K*P] -> [128, nb*K*P//16] dma_gather layout
        flat = a.reshape(-1)
        w = flat.reshape(-1, 16).T  # [16, I/16]
        return np.tile(w, (8, 1)).copy()

    per_core = []
    for c in range(cores):
        dl = dloc[c].reshape(nb, K0 + K1, P).transpose(2, 0, 1).reshape(P, -1)
        inv = np.ones(nloc_pad, dtype=np.float32)
        inv[:nloc] = invdeg[c * nloc : (c + 1) * nloc]
        per_core.append(
            dict(
                idx0=wrap_idx(idx0[c]),
                idx1=wrap_idx(idx1[c]),
                dloc=np.ascontiguousarray(dl),
                invd=np.broadcast_to(inv[None, :], (P, nloc_pad)).copy(),
            )
        )
    return per_core, K0, K1, invdeg


def build_nc(n_nodes, cores, K0, K1, G, iters=1):
    import os
    abl = set(os.environ.get("BASS_ABL", "").split(","))
    SP = False       # single_packet=True hangs this runtime ucode (re-confirmed)
    NQ = 4           # spread gathers over all 4 SWDGE queue pairs
    FINE = int(os.environ.get("BASS_FINE", "9"))  # gather calls per (group, half)
    STB = int(os.environ.get("BASS_STB", "2"))
    PADNEG = False   # trailing -1 idx corrupts results on this ucode
    TABLOCAL = False
    OHG = int(os.environ.get("BASS_OHG", "6"))  # one-hot chunks per DVE op (0 = block)
    QRR = os.environ.get("BASS_QRR", "0") == "1"  # global round-robin queue pick
    if PADNEG:
        assert FINE == 7, "trailing -1 pads require one call per (block, half)"
    nloc, nb, nloc_pad, npad, nhalf = _plan(n_nodes, cores)
    assert nb % G == 0
    ngroups = nb // G
    KT = K0 + K1

    nc = bacc.Bacc("TRN2", target_bir_lowering=False, debug=False, num_devices=cores, num_swdge_queues=NQ)

    xT_d = nc.dram_tensor("xT", [P, nloc_pad], dt.float16, kind="ExternalInput").ap()
    idx0_d = nc.dram_tensor("idx0", [P, nb * K0 * P // 16], dt.int16, kind="ExternalInput").ap()
    idx1_d = nc.dram_tensor("idx1", [P, nb * K1 * P // 16], dt.int16, kind="ExternalInput").ap()
    dloc_d = nc.dram_tensor("dloc", [P, nb * KT], dt.float16, kind="ExternalInput").ap()
    invd_d = nc.dram_tensor("invd", [P, nloc_pad], dt.float32, kind="ExternalInput").ap()
    wdram = {
        n: nc.dram_tensor(n, [P, D], dt.float16, kind="ExternalInput").ap()
        for n in ["Wp1T", "Wl1T", "Wr1T", "Wp2T", "Wl2T", "Wr2T"]
    }
    ones1_d = nc.dram_tensor("ones1", [1, P], dt.float16, kind="ExternalInput").ap()
    bp1r_d = nc.dram_tensor("bp1r", [1, D], dt.float16, kind="ExternalInput").ap()
    bp2r_d = nc.dram_tensor("bp2r", [1, D], dt.float16, kind="ExternalInput").ap()
    bl1c_d = nc.dram_tensor("bl1c", [P, 1], dt.float32, kind="ExternalInput").ap()
    bl2r_d = nc.dram_tensor("bl2r", [1, D], dt.float16, kind="ExternalInput").ap()
    iota_d = nc.dram_tensor("iota", [P, P], dt.float16, kind="ExternalInput").ap()

    out_own = nc.dram_tensor("out_own", [nloc_pad, D], dt.float32, kind="ExternalOutput").ap()
    h1own = nc.dram_tensor("h1own", [nloc_pad, D], dt.float16).ap()
    h2own = nc.dram_tensor("h2own", [nloc_pad, D], dt.float16).ap()
    tspace = "Local" if TABLOCAL else "Shared"
    table1 = nc.dram_tensor("table1", [npad, D], dt.float16, addr_space=tspace).ap()
    table2 = nc.dram_tensor("table2", [npad, D], dt.float16, addr_space=tspace).ap()

    groups_all = [list(range(cores))]

    with tile.TileContext(nc) as tc, ExitStack() as ctx:
        const = ctx.enter_context(tc.tile_pool(name="const", bufs=1))
        persist = ctx.enter_context(tc.tile_pool(name="persist", bufs=1))
        stage_p = ctx.enter_context(tc.tile_pool(name="stage", bufs=STB))
        work = ctx.enter_context(tc.tile_pool(name="work", bufs=3))
        _ohg = KT if OHG == 0 else OHG
        ohp = ctx.enter_context(
            tc.tile_pool(name="oh", bufs=3 * math.ceil(KT / _ohg))
        )
        aggsb = ctx.enter_context(tc.tile_pool(name="aggsb", bufs=2))
        outp = ctx.enter_context(tc.tile_pool(name="outp", bufs=3))
        psum_agg = ctx.enter_context(tc.tile_pool(name="psum_agg", bufs=4, space="PSUM"))
        psum_mm = ctx.enter_context(tc.tile_pool(name="psum_mm", bufs=3, space="PSUM"))

        nc.gpsimd.load_library(library_config.mlp)

        def cload(ap_dram, shape, dtype, tag):
            t = const.tile(shape, dtype, tag=tag)
            nc.sync.dma_start(t[:], ap_dram)
            return t

        wsb = {n: cload(wdram[n][:, :], [P, D], dt.float16, n) for n in wdram}
        ones1 = cload(ones1_d[:, :], [1, P], dt.float16, "ones1")
        bp1r = cload(bp1r_d[:, :], [1, D], dt.float16, "bp1r")
        bp2r = cload(bp2r_d[:, :], [1, D], dt.float16, "bp2r")
        bl1c = cload(bl1c_d[:, :], [P, 1], dt.float32, "bl1c")
        bl2r = cload(bl2r_d[:, :], [1, D], dt.float16, "bl2r")
        iota = cload(iota_d[:, :], [P, P], dt.float16, "iota")
        dloc_sb = cload(dloc_d[:, :], [P, nb * KT], dt.float16, "dloc")
        invd_sb = cload(invd_d[:, :], [P, nloc_pad], dt.float32, "invd")
        idx0_sb = cload(idx0_d[:, :], [P, nb * K0 * P // 16], dt.int16, "idx0")
        idx1_sb = cload(idx1_d[:, :], [P, nb * K1 * P // 16], dt.int16, "idx1")

        xT_sb = cload(xT_d[:, :], [P, nloc_pad], dt.float16, "xT")
        h1T_sb = persist.tile([P, nloc_pad], dt.float16, tag="h1T")

        _abl_st = None
        if "nogather" in abl and "noagg" not in abl:
            _ast0 = persist.tile([P, G * K0, D], dt.float16, tag="ast0")
            _ast1 = persist.tile([P, G * K1, D], dt.float16, tag="ast1")
            _m0 = nc.vector.memset(_ast0[:], 0.25)
            _m1 = nc.vector.memset(_ast1[:], 0.25)
            _abl_st = (_ast0, _ast1)

        if PADNEG:
            for _i in range(STB):
                _pst0 = stage_p.tile([P, G * K0, D], dt.float16, tag="st0")
                _pm0 = nc.vector.memset(_pst0[:], 0.25)
                _pst1 = stage_p.tile([P, G * K1, D], dt.float16, tag="st1")
                _pm1 = nc.vector.memset(_pst1[:], 0.25)

        _qctr = [0]

        def _next_q(default):
            if not QRR:
                return default
            q = _qctr[0] % NQ
            _qctr[0] += 1
            return q

        def _phase_a():
            # ---------------- Phase A: layer-1 projection of own rows ----------
            for b in range(nb):
                sl = slice(b * P, (b + 1) * P)
                p_ps = psum_mm.tile([P, D], dt.float32, tag="mm")
                nc.tensor.matmul(p_ps[:], lhsT=ones1[:], rhs=bp1r[:], start=True, stop=False)
                nc.tensor.matmul(p_ps[:], lhsT=xT_sb[:, sl], rhs=wsb["Wp1T"][:], start=False, stop=True)
                pr = outp.tile([P, D], dt.float16, tag="pr")
                nc.scalar.activation(pr[:], p_ps[:], AF.Relu)
                nc.sync.dma_start(h1own[sl, :], pr[:])

            if "nocc" not in abl:
                nc.gpsimd.collective_compute(
                    "AllGather", OP.bypass, replica_groups=groups_all,
                    ins=[h1own[:, :]], outs=[table1[:, :]],
                )

        def _iter_body(last):
            # ---------------- message+aggregate for one layer -------------------
            def agg_layer(table, root_sb, WlT, WrT, layer):
                for g in range(ngroups):
                    if "noagg" in abl and "nogather" in abl:
                        continue
                    if "nogather" in abl:
                        st0 = _abl_st[0]
                        st1 = _abl_st[1]
                    else:
                        st0 = stage_p.tile([P, G * K0, D], dt.float16, tag="st0")
                        st1 = stage_p.tile([P, G * K1, D], dt.float16, tag="st1")
                        c0 = G * K0 * P // 16
                        c1 = G * K1 * P // 16
                        assert (G * K0) % FINE == 0 and (G * K1) % FINE == 0
                        for f in range(FINE):
                            f0 = c0 // FINE
                            nc.gpsimd.dma_gather(
                                st0[:, f * (G * K0) // FINE : (f + 1) * (G * K0) // FINE, :],
                                table[0:nhalf, :],
                                idx0_sb[:, g * c0 + f * f0 : g * c0 + (f + 1) * f0],
                                G * K0 * P // FINE, G * K0 * P // FINE, D, single_packet=SP,
                                queue_num=_next_q((2 * g * FINE + 2 * f) % NQ),
                            )
                            f1 = c1 // FINE
                            nc.gpsimd.dma_gather(
                                st1[:, f * (G * K1) // FINE : (f + 1) * (G * K1) // FINE, :],
                                table[nhalf:npad, :],
                                idx1_sb[:, g * c1 + f * f1 : g * c1 + (f + 1) * f1],
                                G * K1 * P // FINE, G * K1 * P // FINE, D, single_packet=SP,
                                queue_num=_next_q((2 * g * FINE + 2 * f + 1) % NQ),
                            )
                    if "noagg" in abl:
                        continue
                    for bb in range(G):
                        b = g * G + bb
                        sl = slice(b * P, (b + 1) * P)
                        agg_ps = psum_agg.tile([P, P], dt.float32)
                        # one-hots in few large DVE ops per block: HW per-op
                        # overhead (~250ns) dominates, so amortize it
                        ohg = KT if OHG == 0 else OHG
                        ohs = []
                        for t0 in range(0, KT, ohg):
                            nh = min(ohg, KT - t0)
                            ohw = ohp.tile([P, ohg, P], dt.float16, tag="ohw")
                            nc.vector.tensor_tensor(
                                out=ohw[:, 0:nh, :],
                                in0=dloc_sb[:, b * KT + t0 : b * KT + t0 + nh]
                                .unsqueeze(2)
                                .to_broadcast([P, nh, P]),
                                in1=iota[:].unsqueeze(1).to_broadcast([P, nh, P]),
                                op=OP.is_equal,
                            )
                            ohs.append(ohw)
                        for t in range(KT):
                            oh = ohs[t // ohg][:, t % ohg, :]
                            msg = st0[:, bb * K0 + t, :] if t < K0 else st1[:, bb * K1 + (t - K0), :]
                            nc.tensor.matmul(
                                agg_ps[:], lhsT=msg, rhs=oh,
                                start=(t == 0), stop=(t == KT - 1),
                            )
                        aggT = aggsb.tile([P, P], dt.float16)
                        nc.vector.tensor_tensor(
                            out=aggT[:], in0=agg_ps[:], in1=invd_sb[:, sl], op=OP.mult
                        )
                        if layer == 1:
                            o_ps = psum_mm.tile([P, P], dt.float32, tag="mm")
                            nc.tensor.matmul(o_ps[:], lhsT=WlT[:], rhs=aggT[:], start=True, stop=False)
                            nc.tensor.matmul(o_ps[:], lhsT=WrT[:], rhs=root_sb[:, sl], start=False, stop=True)
                            nc.scalar.activation(h1T_sb[:, sl], o_ps[:], AF.Relu, bias=bl1c[:], scale=1.0)
                            # fused layer-2 projection of this block
                            ps2 = psum_mm.tile([P, D], dt.float32, tag="mm")
                            nc.tensor.matmul(ps2[:], lhsT=ones1[:], rhs=bp2r[:], start=True, stop=False)
                            nc.tensor.matmul(ps2[:], lhsT=h1T_sb[:, sl], rhs=wsb["Wp2T"][:], start=False, stop=True)
                            pr2 = outp.tile([P, D], dt.float16, tag="pr")
                            nc.scalar.activation(pr2[:], ps2[:], AF.Relu)
                            nc.sync.dma_start(h2own[sl, :], pr2[:])
                        else:
                            o_ps = psum_mm.tile([P, D], dt.float32, tag="mm")
                            nc.tensor.matmul(o_ps[:], lhsT=ones1[:], rhs=bl2r[:], start=True, stop=False)
                            nc.tensor.matmul(o_ps[:], lhsT=aggT[:], rhs=WlT[:], start=False, stop=False)
                            nc.tensor.matmul(o_ps[:], lhsT=root_sb[:, sl], rhs=WrT[:], start=False, stop=True)
                            ob = outp.tile([P, D], dt.float32, tag="ob")
                            nc.scalar.activation(ob[:], o_ps[:], AF.Copy)
                            nc.sync.dma_start(out_own[sl, :], ob[:])

            # ---------------- Phase B: layer-1 aggregate -> h1T -----------------
            agg_layer(table1, xT_sb, wsb["Wl1T"], wsb["Wr1T"], layer=1)

            if "nocc" not in abl:
                nc.gpsimd.collective_compute(
                    "AllGather", OP.bypass, replica_groups=groups_all,
                    ins=[h2own[:, :]], outs=[table2[:, :]],
                )

            # software pipelining: next iteration's projection + layer-1
            # AllGather issue here so the AG1 transfer overlaps this
            # iteration's layer-2 aggregation (h1own/table1 WARs are long
            # resolved by now; Tile enforces the rest)
            if not last:
                _phase_a()

            # ---------------- Phase D: layer-2 aggregate -> out ----------------
            agg_layer(table2, xT_sb if "noagg" in abl else h1T_sb, wsb["Wl2T"], wsb["Wr2T"], layer=2)

        _phase_a()
        for i in range(iters):
            _iter_body(last=(i == iters - 1))

    nc.compile()
    return nc


def make_in_maps(inputs, per_core, n_nodes, cores):
    nloc, nb, nloc_pad, npad, nhalf = _plan(n_nodes, cores)
    x = np.asarray(inputs["x"], dtype=np.float32)
    consts = dict(
        Wp1T=np.asarray(inputs["Wp1"]).T.astype(np.float16),
        Wl1T=np.asarray(inputs["Wl1"]).T.astype(np.float16),
        Wr1T=np.asarray(inputs["Wr1"]).T.astype(np.float16),
        Wp2T=np.asarray(inputs["Wp2"]).T.astype(np.float16),
        Wl2T=np.asarray(inputs["Wl2"]).T.astype(np.float16),
        Wr2T=np.asarray(inputs["Wr2"]).T.astype(np.float16),
        ones1=np.ones((1, P), dtype=np.float16),
        bp1r=np.asarray(inputs["bp1"], np.float32).reshape(1, D).astype(np.float16),
        bp2r=np.asarray(inputs["bp2"], np.float32).reshape(1, D).astype(np.float16),
        bl1c=np.asarray(inputs["bl1"], np.float32).reshape(P, 1).copy(),
        bl2r=np.asarray(inputs["bl2"], np.float32).reshape(1, D).astype(np.float16),
        iota=np.broadcast_to(np.arange(P, dtype=np.float16)[None, :], (P, P)).copy(),
    )
    in_maps = []
    for c in range(cores):
        xo = np.zeros((nloc_pad, D), dtype=np.float32)
        xo[:nloc] = x[c * nloc : (c + 1) * nloc]
        m = dict(consts)
        m["xT"] = np.ascontiguousarray(xo.T).astype(np.float16)
        m.update(per_core[c])
        in_maps.append(m)
    return in_maps


_BUILT = {}


def _run(inputs, n_nodes, n_edges, cores, G, trace=False):
    per_core, K0, K1, _ = preprocess(inputs["edge_index"], n_nodes, cores)
    key = (n_nodes, cores, K0, K1, G)
    if key not in _BUILT:
        _BUILT[key] = build_nc(n_nodes, cores, K0, K1, G)
    nc = _BUILT[key]
    in_maps = make_in_maps(inputs, per_core, n_nodes, cores)
    res = run_bass_kernel_spmd(nc, in_maps, list(range(cores)), trace=trace)
    nloc, nb, nloc_pad, npad, nhalf = _plan(n_nodes, cores)
    out = np.concatenate([res.results[c]["out_own"][:nloc] for c in range(cores)], axis=0)
    return out.astype(np.float32), res


def kernel(**inputs):
    out, _ = _run(inputs, N_NODES, N_EDGES, CORES, G=7)
    return out



# revision 31
# speedup vs baseline: 2.7338x; 1.0315x over previous
"""2-layer GraphSAGE on 8 trn2 NeuronCores - v4.

v3 dataflow (HBM-table dma_gather + one-hot scatter matmuls, 4 SWDGE queues,
pre-transposed fp16 xT, bias-fold matmuls, fused layer-2 projection) plus:
  - one-hots built in a few large broadcast is_equal ops per block (OHG=6,
    3 DVE ops/block): HW per-instruction overhead (~250ns) dominates, so
    fewer, larger DVE ops beat many small ones,
  - gather split into 9 calls per (group, half) (FINE=9),
  - psum_mm triple-buffered for cross-phase overlap.

Strategy (graph/data parallel, hardcoded for N=50000, E=800000, D=128, 8 cores):
  - Nodes sharded by contiguous ranges of 6250 (padded to 6272 = 49*128) per core.
  - Host preprocesses edges: sorted by (dst core, dst block, src half, src),
    padded so every (block, half) has a uniform chunk count across cores (SPMD).
  - Device per layer:
      * project own rows: p = relu(x @ WpT + bp)  -> fp16, AllGather into a
        replicated [50176,128] fp16 table in DRAM.
      * dma_gather (SWDGE) message rows from the table (two int16-indexed
        halves), 128 edges per chunk.
      * scatter via one-hot matmuls: aggT[k,d] += msg[e,k]^T @ onehot[e,d],
        onehot built on DVE with is_equal against an iota tile.
      * mean via per-dst invdeg multiply, then output matmuls + bias (+relu).
  - Layer-2 output rows are written per core and concatenated on host.
"""

import math
from contextlib import ExitStack

import numpy as np

import concourse.bacc as bacc
import concourse.bass as bass
import concourse.tile as tile
from concourse import library_config, mybir
from concourse.bass_utils import run_bass_kernel_spmd

P = 128
D = 128
CORES = 8
N_NODES = 50000
N_EDGES = 800000

AF = mybir.ActivationFunctionType
OP = mybir.AluOpType
dt = mybir.dt


def _plan(n_nodes, cores):
    nloc = n_nodes // cores
    assert nloc * cores == n_nodes
    nb = math.ceil(nloc / P)
    nloc_pad = nb * P
    npad = cores * nloc_pad
    nhalf = npad // 2
    assert nhalf < 32768, "dma_gather idx is int16"
    return nloc, nb, nloc_pad, npad, nhalf


def preprocess(edge_index, n_nodes, cores):
    """Returns per-core gather/scatter metadata + uniform chunk counts K0, K1."""
    nloc, nb, nloc_pad, npad, nhalf = _plan(n_nodes, cores)
    src = np.asarray(edge_index[0], dtype=np.int64)
    dst = np.asarray(edge_index[1], dtype=np.int64)
    E = src.shape[0]

    deg = np.bincount(dst, minlength=n_nodes).astype(np.float64)
    invdeg = (1.0 / np.maximum(deg, 1.0)).astype(np.float32)

    csrc = src // nloc
    r_src = csrc * nloc_pad + (src - csrc * nloc)  # padded row id of source
    half = (r_src >= nhalf).astype(np.int64)
    idx_in_half = (r_src - half * nhalf).astype(np.int64)

    cdst = dst // nloc
    ldst = dst - cdst * nloc
    blk = ldst // P
    dblk = ldst % P

    # sort edges by (dst core, dst block, src half, src row) — src order gives
    # the DMA engines ascending-address locality within each gather list
    order = np.lexsort((idx_in_half, half, blk, cdst))
    s_half = half[order]
    s_idx = idx_in_half[order]
    s_dblk = dblk[order]
    key = ((cdst[order] * nb + blk[order]) * 2 + s_half).astype(np.int64)

    counts = np.bincount(key, minlength=cores * nb * 2)
    starts = np.zeros(cores * nb * 2 + 1, dtype=np.int64)
    np.cumsum(counts, out=starts[1:])
    rank = np.arange(E, dtype=np.int64) - starts[key]

    cnt = counts.reshape(cores, nb, 2)
    K0 = max(1, int(math.ceil(cnt[:, :, 0].max() / P)))
    K1 = max(1, int(math.ceil(cnt[:, :, 1].max() / P)))

    # idx arrays: [cores, nb, K*P] int16 (pad = 0, harmless row gathered,
    # neutralized by dloc pad = 255 in the one-hot); dloc: [cores, nb, (K0+K1)*P]
    fill = 0
    idx0 = np.full((cores, nb, K0 * P), fill, dtype=np.int16)
    idx1 = np.full((cores, nb, K1 * P), fill, dtype=np.int16)
    dloc = np.full((cores, nb, (K0 + K1) * P), 255.0, dtype=np.float16)

    core_k = key // (nb * 2)
    blk_k = (key // 2) % nb
    m0 = s_half == 0
    m1 = ~m0
    idx0[core_k[m0], blk_k[m0], rank[m0]] = s_idx[m0].astype(np.int16)
    idx1[core_k[m1], blk_k[m1], rank[m1]] = s_idx[m1].astype(np.int16)
    dloc[core_k[m0], blk_k[m0], rank[m0]] = s_dblk[m0].astype(np.float16)
    dloc[core_k[m1], blk_k[m1], K0 * P + rank[m1]] = s_dblk[m1].astype(np.float16)

    def wrap_idx(a):  # [nb, K*P] -> [128, nb*K*P//16] dma_gather layout
        flat = a.reshape(-1)
        w = flat.reshape(-1, 16).T  # [16, I/16]
        return np.tile(w, (8, 1)).copy()

    per_core = []
    for c in range(cores):
        dl = dloc[c].reshape(nb, K0 + K1, P).transpose(2, 0, 1).reshape(P, -1)
        inv = np.ones(nloc_pad, dtype=np.float32)
        inv[:nloc] = invdeg[c * nloc : (c + 1) * nloc]
        per_core.append(
            dict(
                idx0=wrap_idx(idx0[c]),
                idx1=wrap_idx(idx1[c]),
                dloc=np.ascontiguousarray(dl),
                invd=np.broadcast_to(inv[None, :], (P, nloc_pad)).copy(),
            )
        )
    return per_core, K0, K1, invdeg


def build_nc(n_nodes, cores, K0, K1, G, iters=1):
    import os
    abl = set(os.environ.get("BASS_ABL", "").split(","))
    SP = False       # single_packet=True hangs this runtime ucode (re-confirmed)
    NQ = 4           # spread gathers over all 4 SWDGE queue pairs
    FINE = int(os.environ.get("BASS_FINE", "9"))  # gather calls per (group, half)
    STB = int(os.environ.get("BASS_STB", "2"))
    PADNEG = False   # trailing -1 idx corrupts results on this ucode
    TABLOCAL = False
    OHG = int(os.environ.get("BASS_OHG", "6"))  # one-hot chunks per DVE op (0 = block)
    QRR = os.environ.get("BASS_QRR", "0") == "1"  # global round-robin queue pick
    if PADNEG:
        assert FINE == 7, "trailing -1 pads require one call per (block, half)"
    nloc, nb, nloc_pad, npad, nhalf = _plan(n_nodes, cores)
    assert nb % G == 0
    ngroups = nb // G
    KT = K0 + K1

    nc = bacc.Bacc("TRN2", target_bir_lowering=False, debug=False, num_devices=cores, num_swdge_queues=NQ)

    xT_d = nc.dram_tensor("xT", [P, nloc_pad], dt.float16, kind="ExternalInput").ap()
    idx0_d = nc.dram_tensor("idx0", [P, nb * K0 * P // 16], dt.int16, kind="ExternalInput").ap()
    idx1_d = nc.dram_tensor("idx1", [P, nb * K1 * P // 16], dt.int16, kind="ExternalInput").ap()
    dloc_d = nc.dram_tensor("dloc", [P, nb * KT], dt.float16, kind="ExternalInput").ap()
    invd_d = nc.dram_tensor("invd", [P, nloc_pad], dt.float32, kind="ExternalInput").ap()
    wdram = {
        n: nc.dram_tensor(n, [P, D], dt.float16, kind="ExternalInput").ap()
        for n in ["Wp1T", "Wl1T", "Wr1T", "Wp2T", "Wl2T", "Wr2T"]
    }
    ones1_d = nc.dram_tensor("ones1", [1, P], dt.float16, kind="ExternalInput").ap()
    bp1r_d = nc.dram_tensor("bp1r", [1, D], dt.float16, kind="ExternalInput").ap()
    bp2r_d = nc.dram_tensor("bp2r", [1, D], dt.float16, kind="ExternalInput").ap()
    bl1c_d = nc.dram_tensor("bl1c", [P, 1], dt.float32, kind="ExternalInput").ap()
    bl2r_d = nc.dram_tensor("bl2r", [1, D], dt.float16, kind="ExternalInput").ap()
    iota_d = nc.dram_tensor("iota", [P, P], dt.float16, kind="ExternalInput").ap()

    out_own = nc.dram_tensor("out_own", [nloc_pad, D], dt.float32, kind="ExternalOutput").ap()
    h1own = nc.dram_tensor("h1own", [nloc_pad, D], dt.float16).ap()
    h2own = nc.dram_tensor("h2own", [nloc_pad, D], dt.float16).ap()
    tspace = "Local" if TABLOCAL else "Shared"
    table1 = nc.dram_tensor("table1", [npad, D], dt.float16, addr_space=tspace).ap()
    table2 = nc.dram_tensor("table2", [npad, D], dt.float16, addr_space=tspace).ap()

    groups_all = [list(range(cores))]

    with tile.TileContext(nc) as tc, ExitStack() as ctx:
        const = ctx.enter_context(tc.tile_pool(name="const", bufs=1))
        persist = ctx.enter_context(tc.tile_pool(name="persist", bufs=1))
        stage_p = ctx.enter_context(tc.tile_pool(name="stage", bufs=STB))
        work = ctx.enter_context(tc.tile_pool(name="work", bufs=3))
        _ohg = KT if OHG == 0 else OHG
        ohp = ctx.enter_context(
            tc.tile_pool(name="oh", bufs=3 * math.ceil(KT / _ohg))
        )
        aggsb = ctx.enter_context(tc.tile_pool(name="aggsb", bufs=4))
        outp = ctx.enter_context(tc.tile_pool(name="outp", bufs=5))
        psum_agg = ctx.enter_context(tc.tile_pool(name="psum_agg", bufs=5, space="PSUM"))
        psum_mm = ctx.enter_context(tc.tile_pool(name="psum_mm", bufs=3, space="PSUM"))

        nc.gpsimd.load_library(library_config.mlp)

        def cload(ap_dram, shape, dtype, tag):
            t = const.tile(shape, dtype, tag=tag)
            nc.sync.dma_start(t[:], ap_dram)
            return t

        wsb = {n: cload(wdram[n][:, :], [P, D], dt.float16, n) for n in wdram}
        ones1 = cload(ones1_d[:, :], [1, P], dt.float16, "ones1")
        bp1r = cload(bp1r_d[:, :], [1, D], dt.float16, "bp1r")
        bp2r = cload(bp2r_d[:, :], [1, D], dt.float16, "bp2r")
        bl1c = cload(bl1c_d[:, :], [P, 1], dt.float32, "bl1c")
        bl2r = cload(bl2r_d[:, :], [1, D], dt.float16, "bl2r")
        iota = cload(iota_d[:, :], [P, P], dt.float16, "iota")
        dloc_sb = cload(dloc_d[:, :], [P, nb * KT], dt.float16, "dloc")
        invd_sb = cload(invd_d[:, :], [P, nloc_pad], dt.float32, "invd")
        idx0_sb = cload(idx0_d[:, :], [P, nb * K0 * P // 16], dt.int16, "idx0")
        idx1_sb = cload(idx1_d[:, :], [P, nb * K1 * P // 16], dt.int16, "idx1")

        xT_sb = cload(xT_d[:, :], [P, nloc_pad], dt.float16, "xT")
        h1T_sb = persist.tile([P, nloc_pad], dt.float16, tag="h1T")

        _abl_st = None
        if "nogather" in abl and "noagg" not in abl:
            _ast0 = persist.tile([P, G * K0, D], dt.float16, tag="ast0")
            _ast1 = persist.tile([P, G * K1, D], dt.float16, tag="ast1")
            _m0 = nc.vector.memset(_ast0[:], 0.25)
            _m1 = nc.vector.memset(_ast1[:], 0.25)
            _abl_st = (_ast0, _ast1)

        if PADNEG:
            for _i in range(STB):
                _pst0 = stage_p.tile([P, G * K0, D], dt.float16, tag="st0")
                _pm0 = nc.vector.memset(_pst0[:], 0.25)
                _pst1 = stage_p.tile([P, G * K1, D], dt.float16, tag="st1")
                _pm1 = nc.vector.memset(_pst1[:], 0.25)

        _qctr = [0]

        def _next_q(default):
            if not QRR:
                return default
            q = _qctr[0] % NQ
            _qctr[0] += 1
            return q

        def _phase_a():
            # ---------------- Phase A: layer-1 projection of own rows ----------
            for b in range(nb):
                sl = slice(b * P, (b + 1) * P)
                p_ps = psum_mm.tile([P, D], dt.float32, tag="mm")
                nc.tensor.matmul(p_ps[:], lhsT=ones1[:], rhs=bp1r[:], start=True, stop=False)
                nc.tensor.matmul(p_ps[:], lhsT=xT_sb[:, sl], rhs=wsb["Wp1T"][:], start=False, stop=True)
                pr = outp.tile([P, D], dt.float16, tag="pr")
                nc.scalar.activation(pr[:], p_ps[:], AF.Relu)
                nc.sync.dma_start(h1own[sl, :], pr[:])

            if "nocc" not in abl:
                nc.gpsimd.collective_compute(
                    "AllGather", OP.bypass, replica_groups=groups_all,
                    ins=[h1own[:, :]], outs=[table1[:, :]],
                )

        def _iter_body(last):
            # ---------------- message+aggregate for one layer -------------------
            def agg_layer(table, root_sb, WlT, WrT, layer):
                for g in range(ngroups):
                    if "noagg" in abl and "nogather" in abl:
                        continue
                    if "nogather" in abl:
                        st0 = _abl_st[0]
                        st1 = _abl_st[1]
                    else:
                        st0 = stage_p.tile([P, G * K0, D], dt.float16, tag="st0")
                        st1 = stage_p.tile([P, G * K1, D], dt.float16, tag="st1")
                        c0 = G * K0 * P // 16
                        c1 = G * K1 * P // 16
                        assert (G * K0) % FINE == 0 and (G * K1) % FINE == 0
                        for f in range(FINE):
                            f0 = c0 // FINE
                            nc.gpsimd.dma_gather(
                                st0[:, f * (G * K0) // FINE : (f + 1) * (G * K0) // FINE, :],
                                table[0:nhalf, :],
                                idx0_sb[:, g * c0 + f * f0 : g * c0 + (f + 1) * f0],
                                G * K0 * P // FINE, G * K0 * P // FINE, D, single_packet=SP,
                                queue_num=_next_q((2 * g * FINE + 2 * f) % NQ),
                            )
                            f1 = c1 // FINE
                            nc.gpsimd.dma_gather(
                                st1[:, f * (G * K1) // FINE : (f + 1) * (G * K1) // FINE, :],
                                table[nhalf:npad, :],
                                idx1_sb[:, g * c1 + f * f1 : g * c1 + (f + 1) * f1],
                                G * K1 * P // FINE, G * K1 * P // FINE, D, single_packet=SP,
                                queue_num=_next_q((2 * g * FINE + 2 * f + 1) % NQ),
                            )
                    if "noagg" in abl:
                        continue
                    for bb in range(G):
                        b = g * G + bb
                        sl = slice(b * P, (b + 1) * P)
                        agg_ps = psum_agg.tile([P, P], dt.float32)
                        # one-hots in few large DVE ops per block: HW per-op
                        # overhead (~250ns) dominates, so amortize it
                        ohg = KT if OHG == 0 else OHG
                        ohs = []
                        for t0 in range(0, KT, ohg):
                            nh = min(ohg, KT - t0)
                            ohw = ohp.tile([P, ohg, P], dt.float16, tag="ohw")
                            nc.vector.tensor_tensor(
                                out=ohw[:, 0:nh, :],
                                in0=dloc_sb[:, b * KT + t0 : b * KT + t0 + nh]
                                .unsqueeze(2)
                                .to_broadcast([P, nh, P]),
                                in1=iota[:].unsqueeze(1).to_broadcast([P, nh, P]),
                                op=OP.is_equal,
                            )
                            ohs.append(ohw)
                        for t in range(KT):
                            oh = ohs[t // ohg][:, t % ohg, :]
                            msg = st0[:, bb * K0 + t, :] if t < K0 else st1[:, bb * K1 + (t - K0), :]
                            nc.tensor.matmul(
                                agg_ps[:], lhsT=msg, rhs=oh,
                                start=(t == 0), stop=(t == KT - 1),
                            )
                        aggT = aggsb.tile([P, P], dt.float16)
                        nc.vector.tensor_tensor(
                            out=aggT[:], in0=agg_ps[:], in1=invd_sb[:, sl], op=OP.mult
                        )
                        if layer == 1:
                            o_ps = psum_mm.tile([P, P], dt.float32, tag="mm")
                            nc.tensor.matmul(o_ps[:], lhsT=WlT[:], rhs=aggT[:], start=True, stop=False)
                            nc.tensor.matmul(o_ps[:], lhsT=WrT[:], rhs=root_sb[:, sl], start=False, stop=True)
                            nc.scalar.activation(h1T_sb[:, sl], o_ps[:], AF.Relu, bias=bl1c[:], scale=1.0)
                            # fused layer-2 projection of this block
                            ps2 = psum_mm.tile([P, D], dt.float32, tag="mm")
                            nc.tensor.matmul(ps2[:], lhsT=ones1[:], rhs=bp2r[:], start=True, stop=False)
                            nc.tensor.matmul(ps2[:], lhsT=h1T_sb[:, sl], rhs=wsb["Wp2T"][:], start=False, stop=True)
                            pr2 = outp.tile([P, D], dt.float16, tag="pr")
                            nc.scalar.activation(pr2[:], ps2[:], AF.Relu)
                            nc.sync.dma_start(h2own[sl, :], pr2[:])
                        else:
                            o_ps = psum_mm.tile([P, D], dt.float32, tag="mm")
                            nc.tensor.matmul(o_ps[:], lhsT=ones1[:], rhs=bl2r[:], start=True, stop=False)
                            nc.tensor.matmul(o_ps[:], lhsT=aggT[:], rhs=WlT[:], start=False, stop=False)
                            nc.tensor.matmul(o_ps[:], lhsT=root_sb[:, sl], rhs=WrT[:], start=False, stop=True)
                            ob = outp.tile([P, D], dt.float32, tag="ob")
                            nc.scalar.activation(ob[:], o_ps[:], AF.Copy)
                            nc.sync.dma_start(out_own[sl, :], ob[:])

            # ---------------- Phase B: layer-1 aggregate -> h1T -----------------
            agg_layer(table1, xT_sb, wsb["Wl1T"], wsb["Wr1T"], layer=1)

            if "nocc" not in abl:
                nc.gpsimd.collective_compute(
                    "AllGather", OP.bypass, replica_groups=groups_all,
                    ins=[h2own[:, :]], outs=[table2[:, :]],
                )

            # software pipelining: next iteration's projection + layer-1
            # AllGather issue here so the AG1 transfer overlaps this
            # iteration's layer-2 aggregation (h1own/table1 WARs are long
            # resolved by now; Tile enforces the rest)
            if not last:
                _phase_a()

            # ---------------- Phase D: layer-2 aggregate -> out ----------------
            agg_layer(table2, xT_sb if "noagg" in abl else h1T_sb, wsb["Wl2T"], wsb["Wr2T"], layer=2)

        _phase_a()
        for i in range(iters):
            _iter_body(last=(i == iters - 1))

    nc.compile()
    return nc


def make_in_maps(inputs, per_core, n_nodes, cores):
    nloc, nb, nloc_pad, npad, nhalf = _plan(n_nodes, cores)
    x = np.asarray(inputs["x"], dtype=np.float32)
    consts = dict(
        Wp1T=np.asarray(inputs["Wp1"]).T.astype(np.float16),
        Wl1T=np.asarray(inputs["Wl1"]).T.astype(np.float16),
        Wr1T=np.asarray(inputs["Wr1"]).T.astype(np.float16),
        Wp2T=np.asarray(inputs["Wp2"]).T.astype(np.float16),
        Wl2T=np.asarray(inputs["Wl2"]).T.astype(np.float16),
        Wr2T=np.asarray(inputs["Wr2"]).T.astype(np.float16),
        ones1=np.ones((1, P), dtype=np.float16),
        bp1r=np.asarray(inputs["bp1"], np.float32).reshape(1, D).astype(np.float16),
        bp2r=np.asarray(inputs["bp2"], np.float32).reshape(1, D).astype(np.float16),
        bl1c=np.asarray(inputs["bl1"], np.float32).reshape(P, 1).copy(),
        bl2r=np.asarray(inputs["bl2"], np.float32).reshape(1, D).astype(np.float16),
        iota=np.broadcast_to(np.arange(P, dtype=np.float16)[None, :], (P, P)).copy(),
    )
    in_maps = []
    for c in range(cores):
        xo = np.zeros((nloc_pad, D), dtype=np.float32)
        xo[:nloc] = x[c * nloc : (c + 1) * nloc]
        m = dict(consts)
        m["xT"] = np.ascontiguousarray(xo.T).astype(np.float16)
        m.update(per_core[c])
        in_maps.append(m)
    return in_maps


_BUILT = {}


def _run(inputs, n_nodes, n_edges, cores, G, trace=False):
    per_core, K0, K1, _ = preprocess(inputs["edge_index"], n_nodes, cores)
    key = (n_nodes, cores, K0, K1, G)
    if key not in _BUILT:
        _BUILT[key] = build_nc(n_nodes, cores, K0, K1, G)
    nc = _BUILT[key]
    in_maps = make_in_maps(inputs, per_core, n_nodes, cores)
    res = run_bass_kernel_spmd(nc, in_maps, list(range(cores)), trace=trace)
    nloc, nb, nloc_pad, npad, nhalf = _plan(n_nodes, cores)
    out = np.concatenate([res.results[c]["out_own"][:nloc] for c in range(cores)], axis=0)
    return out.astype(np.float32), res


def kernel(**inputs):
    out, _ = _run(inputs, N_NODES, N_EDGES, CORES, G=7)
    return out

